# revision 1
# baseline (speedup 1.0000x reference)
"""Trainium2 Bass kernel for multi-head attention (B=4, N=2048, C=768, H=12).

Sharding: 8 cores = 4 batches x 2 sequence-halves. Each core computes K/V for
its batch's full 2048-token sequence (duplicated across the 2 cores sharing a
batch) and Q/attention/proj for its own 1024 query rows. No collectives; the
host gather is pure concatenation. The host passes x[b].T with the core's own
half rolled to the front, so Q-projection always reads columns 0:1024
(attention is permutation-invariant along keys, so rolling K/V is harmless).

Per head-quad (4 heads) the kernel streams xt from DRAM and projects K/Q/V,
then runs attention for the quad's two head-pairs. Quad q+1's projection
matmuls (PE) overlap quad q's softmax exps (ScalarE), which are the
throughput bottleneck. All matmuls are float32r (1 cyc/row at N>=256).

Attention per (pair, 512-query block, 128-key tile):
  sT[j,i] = kT_h.T @ qT_h        (2 heads row-packed on the 128-deep PE)
  e = exp(SCALE*sT)              (ScalarE, PSUM->SBUF, scale folded in)
  po[0:65,i] += v'[j,0:65].T @ e (v' = [v_h | ones]; row 64 = softmax denom)
normalize: outT = po[0:64] * bcast(1/po[64]).  A pair's two denominators are
  stacked at partitions 0/32 of one tile via tiny SBUF DMAs, one DVE
  reciprocal covers both, then GpSimd partition_broadcast replicates each
  reciprocal row (always sourced from partition 0 — sourcing other
  partitions silently corrupts on hardware) and DVE multiplies.
"""

import os
import ml_dtypes
import numpy as np

B, N, C = 4, 2048, 768
H, HD = 12, 64
SCALE = HD ** -0.5
P = 128
CT = C // P          # 6 contraction tiles
PAIRS = H // 2       # 6 head pairs
QUADS = H // 4       # 3 head quads
IQ = N // 2          # 1024 query rows per core
JT = N // P          # 16 key tiles
TKB = 512            # token-block width streamed from DRAM
NCORES = 8

_cache = {}


def _build_bass():
    import concourse.bass as bass
    import concourse.tile as tile
    import concourse.mybir as mybir
    from concourse import bacc
    from concourse.bass import ts, ds
    from contextlib import ExitStack

    fr = mybir.dt.float32r
    f32 = mybir.dt.float32
    bf16 = mybir.dt.bfloat16
    Exp = mybir.ActivationFunctionType.Exp

    nc = bacc.Bacc("TRN2", target_bir_lowering=False, debug=False)

    xt_d = nc.dram_tensor("xt", [C, N], fr, kind="ExternalInput").ap()
    xtb_d = nc.dram_tensor("xtb", [C, N], bf16, kind="ExternalInput").ap()
    wq_d = nc.dram_tensor("wq", [C, C], fr, kind="ExternalInput").ap()
    wk_d = nc.dram_tensor("wk", [C, C], fr, kind="ExternalInput").ap()
    wv_d = nc.dram_tensor("wv", [C, C], fr, kind="ExternalInput").ap()
    wp_d = nc.dram_tensor("wp", [C, C], fr, kind="ExternalInput").ap()
    bb_d = nc.dram_tensor("bb", [P, C], f32, kind="ExternalInput").ap()
    out_d = nc.dram_tensor("out", [IQ, C], f32, kind="ExternalOutput").ap()

    xt_r = xt_d.rearrange("(o p) n -> p o n", p=P)
    xtb_r = xtb_d.rearrange("(o p) n -> p o n", p=P)
    wq_r = wq_d.rearrange("(o p) n -> p o n", p=P)
    wk_r = wk_d.rearrange("(o p) n -> p o n", p=P)
    wv_r = wv_d.rearrange("(o p) n -> p o n", p=P)
    wp_r = wp_d.rearrange("(o p) n -> p o n", p=P)
    out_r = out_d.rearrange("(t p) n -> t p n", p=P)

    with tile.TileContext(nc) as tc:
        with ExitStack() as ctx:
            persist = ctx.enter_context(tc.tile_pool(name="persist", bufs=1))
            outT_sb = persist.tile([P, PAIRS, IQ], fr, name="outT_sb")
            v_all = persist.tile([P, JT, H * 128], bf16, name="v_all")
            v_all_r = v_all.rearrange("p t (h e) -> p t h e", e=128)
            nc.any.memzero(v_all[:])
            ones_sb = persist.tile([P, 64], fr, name="ones_sb")
            with nc.allow_low_precision(reason="f32r is bitwise f32"):
                nc.vector.tensor_copy(
                    ones_sb[:], nc.const_aps.tensor(1.0, [P, 64], f32)
                )
                nc.vector.tensor_copy(
                    v_all_r[:, :, :, 64],
                    nc.const_aps.tensor(1.0, [P, JT, H], bf16),
                )

            with ExitStack() as qctx:
                wpool = qctx.enter_context(tc.tile_pool(name="wq", bufs=1))
                kvq = qctx.enter_context(tc.tile_pool(name="kvq", bufs=2))
                xt_pool = qctx.enter_context(tc.tile_pool(name="xtp", bufs=2))
                apsum = qctx.enter_context(
                    tc.tile_pool(name="apsum", bufs=2, space="PSUM")
                )
                spsum = qctx.enter_context(
                    tc.tile_pool(name="spsum", bufs=2, space="PSUM")
                )
                opsum = qctx.enter_context(
                    tc.tile_pool(name="opsum", bufs=2, space="PSUM")
                )
                expt_pool = qctx.enter_context(tc.tile_pool(name="expt", bufs=4))
                nrm_pool = qctx.enter_context(tc.tile_pool(name="nrm", bufs=2))
                poS_pool = qctx.enter_context(tc.tile_pool(name="poSp", bufs=4))

                for q in range(QUADS):
                    # ---- load this quad's weight slices ----
                    wq_t = wpool.tile([P, CT, 256], fr, tag="wq_t")
                    nc.sync.dma_start(wq_t[:], wq_r[:, :, ts(q, 256)])
                    wk_t = wpool.tile([P, CT, 256], fr, tag="wk_t")
                    nc.sync.dma_start(wk_t[:], wk_r[:, :, ts(q, 256)])
                    if q == 0:
                        wv_t = wpool.tile([P, CT, 512], fr, tag="wv_t", name="wv_t")
                        nc.sync.dma_start(wv_t[:], wv_r[:, :, 0:512])
                    elif q == 1:
                        wv_t = wpool.tile([P, CT, 256], fr, tag="wv_t", name="wv_t")
                        nc.sync.dma_start(wv_t[:], wv_r[:, :, 512:768])

                    kT_q = kvq.tile([P, 2, N], fr, tag="kT_q")
                    qT_q = kvq.tile([P, 2, IQ], fr, tag="qT_q")


                    # ---- projections (streamed over token blocks) ----
                    for tb in range(N // TKB):
                        xt_t = xt_pool.tile([P, CT, TKB], fr, tag="xt")
                        nc.sync.dma_start(xt_t[:], xt_r[:, :, ts(tb, TKB)])
                        for tl in range(2):
                            ps = apsum.tile([P, TKB], f32, tag="aps")
                            for c in range(CT):
                                nc.tensor.matmul(
                                    ps[:],
                                    wk_t[:, c, ts(tl, P)],
                                    xt_t[:, c, :],
                                    start=(c == 0),
                                    stop=(c == CT - 1),
                                )
                            nc.vector.tensor_copy(kT_q[:, tl, ts(tb, TKB)], ps[:])
                        if tb < IQ // TKB:
                            for tl in range(2):
                                ps = apsum.tile([P, TKB], f32, tag="aps")
                                for c in range(CT):
                                    nc.tensor.matmul(
                                        ps[:],
                                        wq_t[:, c, ts(tl, P)],
                                        xt_t[:, c, :],
                                        start=(c == 0),
                                        stop=(c == CT - 1),
                                    )
                                nc.vector.tensor_copy(
                                    qT_q[:, tl, ts(tb, TKB)], ps[:]
                                )
                        if q < 2:
                            vn = 512 if q == 0 else 256
                            h0 = 0 if q == 0 else 8
                            for tt in range(TKB // P):
                                ps = apsum.tile([P, vn], f32, tag="aps")
                                for c in range(CT):
                                    nc.tensor.matmul(
                                        ps[:],
                                        xt_t[:, c, ts(tt, P)],
                                        wv_t[:, c, 0:vn],
                                        start=(c == 0),
                                        stop=(c == CT - 1),
                                    )
                                gtt = (tb * TKB) // P + tt
                                with nc.allow_low_precision(reason="bf16 value path"):
                                    nc.vector.tensor_copy(
                                        v_all_r[:, gtt, h0 : h0 + vn // 64, 0:64],
                                        ps.rearrange("p (h e) -> p h e", e=64),
                                    )

                    # ---- attention for the quad's two pairs ----
                    for ib in range(IQ // 512):
                        poS_all = []
                        den_all = []
                        for tl in range(2):
                            t = 2 * q + tl
                            den_q = nrm_pool.tile([65, 512], fr, tag="den_q")
                            with nc.allow_low_precision(reason="f32r is bitwise f32"):
                                nc.vector.tensor_copy(
                                    den_q[:], nc.const_aps.tensor(1.0, [65, 512], f32)
                                )
                            den_all.append(den_q)
                            po0 = opsum.tile([P, 512], f32, tag="po")
                            po1 = opsum.tile([P, 512], f32, tag="po")
                            pos = (po0, po1)
                            for jt in range(JT):
                                ss = spsum.tile([P, 1024], f32, tag="ss")
                                nc.tensor.matmul(
                                    ss[:, 0:512],
                                    kT_q[0:64, tl, ts(jt, P)],
                                    qT_q[0:64, tl, ts(ib, 512)],
                                    start=True,
                                    stop=True,
                                )
                                nc.tensor.matmul(
                                    ss[:, 512:1024],
                                    kT_q[64:128, tl, ts(jt, P)],
                                    qT_q[64:128, tl, ts(ib, 512)],
                                    start=True,
                                    stop=True,
                                )
                                et = expt_pool.tile([P, 1024], bf16, tag="et")
                                nc.scalar.activation(et[:], ss[:], Exp, scale=SCALE)
                                for hh in range(2):
                                    hg = 2 * t + hh
                                    nc.tensor.matmul(
                                        pos[hh][0:128, :],
                                        v_all[:, jt, hg * 128 : hg * 128 + 128],
                                        et[:, hh * 512 : (hh + 1) * 512],
                                        start=(jt == 0),
                                        stop=(jt == JT - 1),
                                    )
                            for hh in range(2):
                                poS = poS_pool.tile([65, 512], fr, tag="poS")
                                with nc.allow_low_precision(
                                    reason="f32r is bitwise f32"
                                ):
                                    nc.vector.tensor_copy(poS[:], pos[hh][0:65, :])
                                # stack this head's denominator at partition 32*hh
                                nc.sync.dma_start(
                                    den_q[32 * hh : 32 * hh + 1, :], poS[64:65, :]
                                )
                                poS_all.append(poS)
                        # one reciprocal per pair covers both heads
                        rd_all = []
                        for tl in range(2):
                            rd_q = nrm_pool.tile([65, 512], fr, tag="rd_q")
                            with nc.allow_low_precision(reason="f32r is bitwise f32"):
                                nc.vector.reciprocal(rd_q[:], den_all[tl][:])
                            rd_all.append(rd_q)
                        for tl in range(2):
                            t = 2 * q + tl
                            for hh in range(2):
                                poS = poS_all[2 * tl + hh]
                                if hh == 0:
                                    rd_src = rd_all[tl]
                                else:
                                    # relocate head-1's reciprocal to partition
                                    # 0: HW partition_broadcast only sources
                                    # partition 0 correctly
                                    rd_src = nrm_pool.tile(
                                        [1, 512], fr, tag="rd1", name="rd1"
                                    )
                                    nc.sync.dma_start(
                                        rd_src[:], rd_all[tl][32:33, :]
                                    )
                                rb_sb = nrm_pool.tile([64, 512], fr, tag="rb_sb")
                                nc.gpsimd.partition_broadcast(
                                    rb_sb[:], rd_src[0:1, :]
                                )
                                with nc.allow_low_precision(
                                    reason="f32r is bitwise f32"
                                ):
                                    nc.vector.tensor_mul(
                                        outT_sb[
                                            hh * 64 : (hh + 1) * 64, t, ts(ib, 512)
                                        ],
                                        poS[0:64, :],
                                        rb_sb[:],
                                    )

            # ---------------- final projection ----------------
            with ExitStack() as pctx:
                ppool = pctx.enter_context(tc.tile_pool(name="pw", bufs=1))
                wp_sb = ppool.tile([P, CT, C], fr, name="wp_sb")
                nc.sync.dma_start(wp_sb[:], wp_r)
                bias_sb = ppool.tile([P, C], f32, name="bias_sb")
                nc.sync.dma_start(bias_sb[:], bb_d)
                ppsum = pctx.enter_context(
                    tc.tile_pool(name="ppsum", bufs=2, space="PSUM")
                )
                outsb_pool = pctx.enter_context(tc.tile_pool(name="outsb", bufs=2))

                for git in range(IQ // P):
                    ob = outsb_pool.tile([P, C], f32, tag="ob")
                    for n0, n1 in ((0, 512), (512, 768)):
                        pp = ppsum.tile([P, 512], f32, tag="pp")
                        for t in range(PAIRS):
                            nc.tensor.matmul(
                                pp[:, 0 : n1 - n0],
                                outT_sb[:, t, ds(git * P, P)],
                                wp_sb[:, t, n0:n1],
                                start=(t == 0),
                                stop=(t == PAIRS - 1),
                            )
                        nc.vector.tensor_add(
                            ob[:, n0:n1], pp[:, 0 : n1 - n0], bias_sb[:, n0:n1]
                        )
                    nc.sync.dma_start(out_r[git], ob[:])

    nc.compile()
    return nc


def _get_nc():
    if "nc" not in _cache:
        _cache["nc"] = _build_bass()
    return _cache["nc"]


def _prep_in_maps(x, w_qkv, w_proj, b_proj):
    x = np.asarray(x, np.float32)
    w_qkv = np.asarray(w_qkv, np.float32)
    w_proj = np.asarray(w_proj, np.float32)
    b_proj = np.asarray(b_proj, np.float32)

    wq = np.ascontiguousarray(w_qkv[0:C].T)
    wk = np.ascontiguousarray(w_qkv[C : 2 * C].T)
    wv = np.ascontiguousarray(w_qkv[2 * C : 3 * C].T)
    wp = np.ascontiguousarray(w_proj.T)
    bb = np.ascontiguousarray(np.broadcast_to(b_proj[None, :], (P, C)))

    in_maps = []
    for core in range(NCORES):
        b, half = core // 2, core % 2
        xT = x[b].T  # [C, N]
        mine = xT[:, half * IQ : (half + 1) * IQ]
        other = xT[:, (1 - half) * IQ : (2 - half) * IQ]
        xt = np.ascontiguousarray(np.concatenate([mine, other], axis=1))
        xtb = xt.astype(ml_dtypes.bfloat16)
        in_maps.append(
            {"xt": xt, "xtb": xtb, "wq": wq, "wk": wk, "wv": wv, "wp": wp, "bb": bb}
        )
    return in_maps


def run(x, w_qkv, w_proj, b_proj, trace=False):
    from concourse import bass_utils

    nc = _get_nc()
    in_maps = _prep_in_maps(x, w_qkv, w_proj, b_proj)
    br = bass_utils.run_bass_kernel_spmd(
        nc, in_maps, core_ids=list(range(NCORES)), trace=trace
    )
    y = np.empty((B, N, C), np.float32)
    for core in range(NCORES):
        b, half = core // 2, core % 2
        y[b, half * IQ : (half + 1) * IQ, :] = br.results[core]["out"]
    return y, br


def kernel(x, w_qkv, w_proj, b_proj):
    y, _ = run(x, w_qkv, w_proj, b_proj, trace=False)
    return y



# revision 2
# speedup vs baseline: 1.1317x; 1.1317x over previous
"""Trainium2 Bass kernel for multi-head attention (B=4, N=2048, C=768, H=12).

Sharding: 8 cores = 4 batches x 2 sequence-halves. Each core computes K/V for
its batch's full 2048-token sequence (duplicated across the 2 cores sharing a
batch) and Q/attention/proj for its own 1024 query rows. No collectives; the
host gather is pure concatenation. The host passes x[b].T with the core's own
half rolled to the front, so Q-projection always reads columns 0:1024
(attention is permutation-invariant along keys, so rolling K/V is harmless).

v2: all-bf16 datapath (x, weights, K, Q, V, exp(s), out.T are bf16; PSUM and
the exp input stay fp32). bf16 stationary operands get separate LDWEIGHTS
instructions, so the two 64-deep QK matmuls of a head pair run concurrently
as PE row tiles (0,0)/(64,0) — 2x over the serialized fp32r pair. V tiles are
65 columns (64 hd + ones row for the softmax denominator), so nothing reads
uninitialized SBUF and no memzero is needed. Softmax normalization packs all
4 denominators of a (quad, ib) block on partitions 0-3 and does ONE DVE
reciprocal (cost scales with free dim only), then GpSimd partition_broadcast
(always sourced from partition 0 — other partitions silently corrupt) and a
DVE multiply produce outT.
"""

import os
import ml_dtypes
import numpy as np

B, N, C = 4, 2048, 768
H, HD = 12, 64
SCALE = HD ** -0.5
P = 128
CT = C // P          # 6 contraction tiles
PAIRS = H // 2       # 6 head pairs
QUADS = H // 4       # 3 head quads
IQ = N // 2          # 1024 query rows per core
JT = N // P          # 16 key tiles
TKB = 512            # token-block width streamed from DRAM
VW = 72              # per-head stride in v_all (65 used: 64 hd + ones)
NCORES = 8

_cache = {}


def _build_bass():
    import concourse.bass as bass
    import concourse.tile as tile
    import concourse.mybir as mybir
    from concourse import bacc
    from concourse.bass import ts, ds
    from contextlib import ExitStack

    f32 = mybir.dt.float32
    bf16 = mybir.dt.bfloat16
    Exp = mybir.ActivationFunctionType.Exp

    nc = bacc.Bacc("TRN2", target_bir_lowering=False, debug=False)

    xt_d = nc.dram_tensor("xt", [C, N], bf16, kind="ExternalInput").ap()
    wq_d = nc.dram_tensor("wq", [C, C], bf16, kind="ExternalInput").ap()
    wk_d = nc.dram_tensor("wk", [C, C], bf16, kind="ExternalInput").ap()
    wv_d = nc.dram_tensor("wv", [C, C], bf16, kind="ExternalInput").ap()
    wp_d = nc.dram_tensor("wp", [C, C], bf16, kind="ExternalInput").ap()
    bb_d = nc.dram_tensor("bb", [P, C], f32, kind="ExternalInput").ap()
    out_d = nc.dram_tensor("out", [IQ, C], f32, kind="ExternalOutput").ap()

    xt_r = xt_d.rearrange("(o p) n -> p o n", p=P)
    wq_r = wq_d.rearrange("(o p) n -> p o n", p=P)
    wk_r = wk_d.rearrange("(o p) n -> p o n", p=P)
    wv_r = wv_d.rearrange("(o p) n -> p o n", p=P)
    wp_r = wp_d.rearrange("(o p) n -> p o n", p=P)
    out_r = out_d.rearrange("(t p) n -> t p n", p=P)

    with tile.TileContext(nc) as tc:
        with ExitStack() as ctx:
            persist = ctx.enter_context(tc.tile_pool(name="persist", bufs=1))
            outT_sb = persist.tile([P, PAIRS, IQ], bf16, name="outT_sb")
            v_all = persist.tile([P, JT, H * VW], bf16, name="v_all")
            v_all_r = v_all.rearrange("p t (h e) -> p t h e", e=VW)
            with nc.allow_low_precision(reason="ones column"):
                nc.vector.tensor_copy(
                    v_all_r[:, :, :, 64],
                    nc.const_aps.tensor(1.0, [P, JT, H], bf16),
                )

            with ExitStack() as qctx:
                wpool = qctx.enter_context(tc.tile_pool(name="wq", bufs=2))
                kvq = qctx.enter_context(tc.tile_pool(name="kvq", bufs=2))
                xt_pool = qctx.enter_context(tc.tile_pool(name="xtp", bufs=2))
                apsum = qctx.enter_context(
                    tc.tile_pool(name="apsum", bufs=2, space="PSUM")
                )
                spsum = qctx.enter_context(
                    tc.tile_pool(name="spsum", bufs=2, space="PSUM")
                )
                opsum = qctx.enter_context(
                    tc.tile_pool(name="opsum", bufs=2, space="PSUM")
                )
                expt_pool = qctx.enter_context(tc.tile_pool(name="expt", bufs=4))
                nrm_pool = qctx.enter_context(tc.tile_pool(name="nrm", bufs=2))
                poS_pool = qctx.enter_context(tc.tile_pool(name="poSp", bufs=4))

                for q in range(QUADS):
                    # ---- load this quad's weight slices ----
                    wk_t = wpool.tile([P, CT, 256], bf16, tag="wk_t")
                    nc.sync.dma_start(wk_t[:], wk_r[:, :, ts(q, 256)])
                    wq_t = wpool.tile([P, CT, 256], bf16, tag="wq_t")
                    nc.sync.dma_start(wq_t[:], wq_r[:, :, ts(q, 256)])
                    if q == 0:
                        wv_t = wpool.tile([P, CT, 512], bf16, tag="wv_t", name="wv_t")
                        nc.sync.dma_start(wv_t[:], wv_r[:, :, 0:512])
                    elif q == 1:
                        wv_t = wpool.tile([P, CT, 256], bf16, tag="wv_t", name="wv_t")
                        nc.sync.dma_start(wv_t[:], wv_r[:, :, 512:768])

                    kT_q = kvq.tile([P, 2, N], bf16, tag="kT_q")
                    qT_q = kvq.tile([P, 2, IQ], bf16, tag="qT_q")

                    # ---- projections (streamed over token blocks) ----
                    for tb in range(N // TKB):
                        xt_t = xt_pool.tile([P, CT, TKB], bf16, tag="xt")
                        nc.sync.dma_start(xt_t[:], xt_r[:, :, ts(tb, TKB)])
                        for tl in range(2):
                            ps = apsum.tile([P, TKB], f32, tag="aps")
                            for c in range(CT):
                                nc.tensor.matmul(
                                    ps[:],
                                    wk_t[:, c, ts(tl, P)],
                                    xt_t[:, c, :],
                                    start=(c == 0),
                                    stop=(c == CT - 1),
                                )
                            with nc.allow_low_precision(reason="bf16 k path"):
                                nc.vector.tensor_copy(
                                    kT_q[:, tl, ts(tb, TKB)], ps[:]
                                )
                        if tb < IQ // TKB:
                            for tl in range(2):
                                ps = apsum.tile([P, TKB], f32, tag="aps")
                                for c in range(CT):
                                    nc.tensor.matmul(
                                        ps[:],
                                        wq_t[:, c, ts(tl, P)],
                                        xt_t[:, c, :],
                                        start=(c == 0),
                                        stop=(c == CT - 1),
                                    )
                                with nc.allow_low_precision(reason="bf16 q path"):
                                    nc.vector.tensor_copy(
                                        qT_q[:, tl, ts(tb, TKB)], ps[:]
                                    )
                        if q < 2:
                            vn = 512 if q == 0 else 256
                            h0 = 0 if q == 0 else 8
                            for tt in range(TKB // P):
                                ps = apsum.tile([P, vn], f32, tag="aps")
                                for c in range(CT):
                                    nc.tensor.matmul(
                                        ps[:],
                                        xt_t[:, c, ts(tt, P)],
                                        wv_t[:, c, 0:vn],
                                        start=(c == 0),
                                        stop=(c == CT - 1),
                                    )
                                gtt = (tb * TKB) // P + tt
                                with nc.allow_low_precision(reason="bf16 v path"):
                                    nc.vector.tensor_copy(
                                        v_all_r[:, gtt, h0 : h0 + vn // 64, 0:64],
                                        ps.rearrange("p (h e) -> p h e", e=64),
                                    )

                    # ---- attention for the quad's two pairs ----
                    for ib in range(IQ // 512):
                        poS_all = []
                        dpk = nrm_pool.tile([4, 512], f32, tag="dpk")
                        for tl in range(2):
                            t = 2 * q + tl
                            po0 = opsum.tile([P, 512], f32, tag="po")
                            po1 = opsum.tile([P, 512], f32, tag="po")
                            pos = (po0, po1)
                            for jt in range(JT):
                                ss = spsum.tile([P, 1024], f32, tag="ss")
                                nc.tensor.matmul(
                                    ss[:, 0:512],
                                    kT_q[0:64, tl, ts(jt, P)],
                                    qT_q[0:64, tl, ts(ib, 512)],
                                    start=True,
                                    stop=True,
                                )
                                nc.tensor.matmul(
                                    ss[:, 512:1024],
                                    kT_q[64:128, tl, ts(jt, P)],
                                    qT_q[64:128, tl, ts(ib, 512)],
                                    start=True,
                                    stop=True,
                                )
                                et = expt_pool.tile([P, 1024], bf16, tag="et")
                                nc.scalar.activation(et[:], ss[:], Exp, scale=SCALE)
                                for hh in range(2):
                                    hg = 2 * t + hh
                                    nc.tensor.matmul(
                                        pos[hh][0:65, :],
                                        v_all_r[:, jt, hg, 0:65],
                                        et[:, hh * 512 : (hh + 1) * 512],
                                        start=(jt == 0),
                                        stop=(jt == JT - 1),
                                    )
                            for hh in range(2):
                                poS = poS_pool.tile([65, 512], f32, tag="poS")
                                nc.vector.tensor_copy(poS[:], pos[hh][0:65, :])
                                # pack this head's denominator at partition 2*tl+hh
                                idx = 2 * tl + hh
                                nc.sync.dma_start(
                                    dpk[idx : idx + 1, :], poS[64:65, :]
                                )
                                poS_all.append(poS)
                        # one reciprocal per (quad, ib) covers all 4 heads
                        rd_q = nrm_pool.tile([4, 512], f32, tag="rd_q")
                        nc.vector.reciprocal(rd_q[:], dpk[:])
                        for tl in range(2):
                            t = 2 * q + tl
                            for hh in range(2):
                                idx = 2 * tl + hh
                                poS = poS_all[idx]
                                if idx == 0:
                                    rd_src = rd_q
                                else:
                                    # relocate to partition 0: HW
                                    # partition_broadcast only sources
                                    # partition 0 correctly
                                    rd_src = nrm_pool.tile(
                                        [1, 512], f32, tag="rd1", name="rd1"
                                    )
                                    nc.sync.dma_start(
                                        rd_src[:], rd_q[idx : idx + 1, :]
                                    )
                                rb_sb = nrm_pool.tile([64, 512], f32, tag="rb_sb")
                                nc.gpsimd.partition_broadcast(
                                    rb_sb[:], rd_src[0:1, :]
                                )
                                with nc.allow_low_precision(
                                    reason="bf16 out path"
                                ):
                                    nc.vector.tensor_mul(
                                        outT_sb[
                                            hh * 64 : (hh + 1) * 64, t, ts(ib, 512)
                                        ],
                                        poS[0:64, :],
                                        rb_sb[:],
                                    )

            # ---------------- final projection ----------------
            with ExitStack() as pctx:
                ppool = pctx.enter_context(tc.tile_pool(name="pw", bufs=1))
                wp_sb = ppool.tile([P, CT, C], bf16, name="wp_sb")
                nc.sync.dma_start(wp_sb[:], wp_r)
                bias_sb = ppool.tile([P, C], f32, name="bias_sb")
                nc.sync.dma_start(bias_sb[:], bb_d)
                ppsum = pctx.enter_context(
                    tc.tile_pool(name="ppsum", bufs=2, space="PSUM")
                )
                outsb_pool = pctx.enter_context(tc.tile_pool(name="outsb", bufs=2))

                for git in range(IQ // P):
                    ob = outsb_pool.tile([P, C], f32, tag="ob")
                    for n0, n1 in ((0, 512), (512, 768)):
                        pp = ppsum.tile([P, 512], f32, tag="pp")
                        for t in range(PAIRS):
                            nc.tensor.matmul(
                                pp[:, 0 : n1 - n0],
                                outT_sb[:, t, ds(git * P, P)],
                                wp_sb[:, t, n0:n1],
                                start=(t == 0),
                                stop=(t == PAIRS - 1),
                            )
                        nc.vector.tensor_add(
                            ob[:, n0:n1], pp[:, 0 : n1 - n0], bias_sb[:, n0:n1]
                        )
                    nc.sync.dma_start(out_r[git], ob[:])

    nc.compile()
    return nc


def _get_nc():
    if "nc" not in _cache:
        _cache["nc"] = _build_bass()
    return _cache["nc"]


def _prep_in_maps(x, w_qkv, w_proj, b_proj):
    x = np.asarray(x, np.float32)
    w_qkv = np.asarray(w_qkv, np.float32)
    w_proj = np.asarray(w_proj, np.float32)
    b_proj = np.asarray(b_proj, np.float32)

    bf = ml_dtypes.bfloat16
    wq = np.ascontiguousarray(w_qkv[0:C].T).astype(bf)
    wk = np.ascontiguousarray(w_qkv[C : 2 * C].T).astype(bf)
    wv = np.ascontiguousarray(w_qkv[2 * C : 3 * C].T).astype(bf)
    wp = np.ascontiguousarray(w_proj.T).astype(bf)
    bb = np.ascontiguousarray(np.broadcast_to(b_proj[None, :], (P, C)))

    in_maps = []
    for core in range(NCORES):
        b, half = core // 2, core % 2
        xT = x[b].T  # [C, N]
        mine = xT[:, half * IQ : (half + 1) * IQ]
        other = xT[:, (1 - half) * IQ : (2 - half) * IQ]
        xt = np.ascontiguousarray(np.concatenate([mine, other], axis=1)).astype(bf)
        in_maps.append(
            {"xt": xt, "wq": wq, "wk": wk, "wv": wv, "wp": wp, "bb": bb}
        )
    return in_maps


def run(x, w_qkv, w_proj, b_proj, trace=False):
    from concourse import bass_utils

    nc = _get_nc()
    in_maps = _prep_in_maps(x, w_qkv, w_proj, b_proj)
    br = bass_utils.run_bass_kernel_spmd(
        nc, in_maps, core_ids=list(range(NCORES)), trace=trace
    )
    y = np.empty((B, N, C), np.float32)
    for core in range(NCORES):
        b, half = core // 2, core % 2
        y[b, half * IQ : (half + 1) * IQ, :] = br.results[core]["out"]
    return y, br


def kernel(x, w_qkv, w_proj, b_proj):
    y, _ = run(x, w_qkv, w_proj, b_proj, trace=False)
    return y
